# revision 1
# baseline (speedup 1.0000x reference)
"""Trainium2 Bass kernel for nn_DRA_C_65644280152592 (dense_transformer).

Strategy: pure data-parallel over batch B=8 across 8 NeuronCores (one sample
per core). All matmul operands staged/cast to fp16 on host (PE runs fp16 at
full rate with fp32 PSUM accumulation); statistics, softmax, epilogues and
output in fp32.

Per-core pipeline (sample b):
  dec[512,112,112] resident in SBUF as fp16 (12.8 MB).
  Stage 1  patch embed: dlT[196,512] = X^T @ pe_w^T, X k-tiles are strided
           APs straight into the resident decoder (no data rearrangement);
           pe_w streamed from HBM as the moving operand. + pe_b via a K=1
           ones-row matmul.
  Stage 2  attention, transpose-free chain:
           km = trans@wk          [196,512]   (lhsT=transT staged on host)
           vT = wv^T@trans^T      [512,196]
           A  = dlT^T@km          [512,512]
           sim= wq^T@A            [512,512]  (s on partitions, t on free)
           InstanceNorm stats via row-reduce + ones-matmul partition reduce,
           softmax over free dim (exp on ACT with accum_out row sums),
           G  = sim_sm^T@wo as lhsT=sm  [512,512]
           recT = G^T@vT          [512,196]
           FIN = relu(rc'(recT)+b2')  [512,196]  (BN2 folded on host)
  Stage 3  mask conv fused: for each 4-row pixel block,
           out = relu(mc'(dec)+b1') * broadcast(FIN)   (BN1 folded on host)
"""
import sys

sys.path.insert(0, "/opt/trn_rl_repo")

import numpy as np

import concourse.bass as bass
import concourse.mybir as mybir
import concourse.tile as tile
from concourse import bacc
from concourse.bass_utils import run_bass_kernel_spmd

F16 = mybir.dt.float16
F32 = mybir.dt.float32
AF = mybir.ActivationFunctionType
AX = mybir.AxisListType
ALU = mybir.AluOpType

CIN, IMG, P = 512, 112, 8
NPR = 14                  # patches per side
NPAT = NPR * NPR          # 196
DEC = SKIP = 512
EMB = 768
BN_EPS = 1e-3
IN_EPS = 1e-3
N_CORES = 8
SIM_N = float(SKIP * SKIP)


def build_nc(repeat: int = 1, stages: int = 99):
    nc = bacc.Bacc(None)

    dec_d = nc.declare_dram_parameter("dec", [CIN, 64, NPAT], F16, isOutput=False)
    trT_d = nc.declare_dram_parameter("transT", [EMB, NPAT], F16, isOutput=False)
    pew_d = nc.declare_dram_parameter("pew", [256, 128, DEC], F16, isOutput=False)
    wq_d = nc.declare_dram_parameter("wq", [DEC, SKIP], F16, isOutput=False)
    wk_d = nc.declare_dram_parameter("wk", [EMB, SKIP], F16, isOutput=False)
    wv_d = nc.declare_dram_parameter("wv", [EMB, SKIP], F16, isOutput=False)
    wo_d = nc.declare_dram_parameter("wo", [SKIP, SKIP], F16, isOutput=False)
    mcT_d = nc.declare_dram_parameter("mcT", [CIN, SKIP], F16, isOutput=False)
    rcT_d = nc.declare_dram_parameter("rcT", [SKIP, SKIP], F16, isOutput=False)
    peb_d = nc.declare_dram_parameter("peb", [1, DEC], F16, isOutput=False)
    b1_d = nc.declare_dram_parameter("b1", [128, 4], F32, isOutput=False)
    b2_d = nc.declare_dram_parameter("b2", [128, 4], F32, isOutput=False)
    psi_d = nc.declare_dram_parameter("psi", [1, 2], F32, isOutput=False)
    out_d = nc.declare_dram_parameter("out", [SKIP, 64, NPAT], F32, isOutput=True)

    bc_scr = nc.dram_tensor("bc_scr", [1, 2], F32)
    bc_scr_ap = bc_scr[:]
    with tile.TileContext(nc) as tc:
        with tc.tile_pool(name="wts", bufs=1) as wts, \
             tc.tile_pool(name="pewp", bufs=8) as pewp, \
             tc.tile_pool(name="work", bufs=2) as work, \
             tc.tile_pool(name="st3", bufs=3) as st3, \
             tc.tile_pool(name="ph", bufs=1, space="PSUM") as ph, \
             tc.tile_pool(name="ps", bufs=4, space="PSUM") as ps:

            def body():
                # ---- resident loads ----
                ones16 = wts.tile([1, 128], F16, tag="ones16")
                nc.vector.memset(ones16, 1.0)

                peb = wts.tile([1, DEC], F16, tag="peb")
                nc.sync.dma_start(out=peb, in_=peb_d[:])
                b1 = wts.tile([128, 4], F32, tag="b1")
                nc.sync.dma_start(out=b1, in_=b1_d[:])
                b2 = wts.tile([128, 4], F32, tag="b2")
                nc.sync.dma_start(out=b2, in_=b2_d[:])
                psi = wts.tile([1, 2], F32, tag="psi")
                nc.sync.dma_start(out=psi, in_=psi_d[:])

                trT = wts.tile([128, 6, NPAT], F16, tag="trT")
                wk = wts.tile([128, 6, SKIP], F16, tag="wk")
                wv = wts.tile([128, 6, SKIP], F16, tag="wv")
                for kt in range(6):
                    nc.sync.dma_start(out=trT[:, kt, :],
                                      in_=trT_d[kt * 128:(kt + 1) * 128, :])
                    nc.sync.dma_start(out=wk[:, kt, :],
                                      in_=wk_d[kt * 128:(kt + 1) * 128, :])
                    nc.sync.dma_start(out=wv[:, kt, :],
                                      in_=wv_d[kt * 128:(kt + 1) * 128, :])
                wq = wts.tile([128, 4, SKIP], F16, tag="wq")
                wo = wts.tile([128, 4, SKIP], F16, tag="wo")
                mcT = wts.tile([128, 4, SKIP], F16, tag="mcT")
                rcT = wts.tile([128, 4, SKIP], F16, tag="rcT")

                def load_late_weights():
                    for kt in range(4):
                        nc.sync.dma_start(out=wq[:, kt, :],
                                          in_=wq_d[kt * 128:(kt + 1) * 128, :])
                        nc.sync.dma_start(out=wo[:, kt, :],
                                          in_=wo_d[kt * 128:(kt + 1) * 128, :])
                        nc.sync.dma_start(out=mcT[:, kt, :],
                                          in_=mcT_d[kt * 128:(kt + 1) * 128, :])
                        nc.sync.dma_start(out=rcT[:, kt, :],
                                          in_=rcT_d[kt * 128:(kt + 1) * 128, :])

                dec_sb = []
                for cb in range(4):
                    t = wts.tile([128, 64 * NPAT], F16, tag=f"dec{cb}",
                                 name=f"dec{cb}")
                    dec_sb.append(t)

                def load_dec(cb):
                    # two half-loads to spread HWDGE queues
                    v = dec_sb[cb].rearrange("p (a b) -> p a b", b=NPAT)
                    nc.sync.dma_start(
                        out=v[:, 0:32, :],
                        in_=dec_d[cb * 128:(cb + 1) * 128, 0:32, :])
                    nc.sync.dma_start(
                        out=v[:, 32:64, :],
                        in_=dec_d[cb * 128:(cb + 1) * 128, 32:64, :])

                load_dec(0)

                if stages < 1:
                    return
                # ---- early attention matmuls (only need trans + wk/wv) ----
                # km[n,s] = sum_e trans[n,e] wk[e,s] ; two M halves of 98
                km = [wts.tile([98, SKIP], F16, tag=f"km{h}", name=f"km{h}")
                      for h in range(2)]
                for h in range(2):
                    pk = ps.tile([98, SKIP], F32, tag="pt")
                    for kt in range(6):
                        nc.tensor.matmul(pk, trT[:, kt, h * 98:(h + 1) * 98],
                                         wk[:, kt, :],
                                         start=(kt == 0), stop=(kt == 5))
                    nc.scalar.copy(km[h], pk)

                # vT[t,n] = sum_e wv[e,t] trans[n,e]  -> [512,196]
                vT = wts.tile([128, 4, NPAT], F16, tag="vT")
                for m in range(4):
                    pv = ps.tile([128, NPAT], F32, tag="pt")
                    for kt in range(6):
                        nc.tensor.matmul(pv, wv[:, kt, m * 128:(m + 1) * 128],
                                         trT[:, kt, :],
                                         start=(kt == 0), stop=(kt == 5))
                    nc.scalar.copy(vT[:, m, :], pv)

                if stages < 2:
                    return
                # ---- stage 1: patch embedding ----
                # dlT[n,d] = sum_{c,py,px} dec[c, 8pr+py, 8pc+px] pew[(py,px,c),d]
                pdl = [ph.tile([98, DEC], F32, tag=f"pdl{h}", name=f"pdl{h}")
                       for h in range(2)]
                for cb in range(4):
                    if cb + 1 < 4:
                        load_dec(cb + 1)
                    if cb == 2:
                        load_late_weights()
                    for pp in range(64):
                        k = cb * 64 + pp
                        pw = pewp.tile([128, DEC], F16, tag="pw")
                        nc.sync.dma_start(out=pw, in_=pew_d[k, :, :])
                        for h in range(2):
                            xs = dec_sb[cb][:, pp * NPAT + 98 * h:
                                            pp * NPAT + 98 * (h + 1)]
                            nc.tensor.matmul(pdl[h], xs, pw,
                                             start=(k == 0), stop=False)
                dlT = [wts.tile([98, DEC], F16, tag=f"dlT{h}", name=f"dlT{h}")
                       for h in range(2)]
                for h in range(2):
                    nc.tensor.matmul(pdl[h], ones16[:1, :98], peb,
                                     start=False, stop=True)
                    nc.scalar.copy(dlT[h], pdl[h])

                if stages < 3:
                    return
                # ---- stage 2: attention ----
                # A[d,t] = sum_n dlT[n,d] km[n,t]
                A = wts.tile([128, 4, SKIP], F16, tag="A")
                for m in range(4):
                    pa = ps.tile([128, SKIP], F32, tag="pt")
                    for h in range(2):
                        nc.tensor.matmul(pa, dlT[h][:, m * 128:(m + 1) * 128],
                                         km[h], start=(h == 0), stop=(h == 1))
                    nc.scalar.copy(A[:, m, :], pa)

                # sim[s,t] = sum_d wq[d,s] A[d,t]
                simf = wts.tile([128, 4, SKIP], F32, tag="simf")
                for m in range(4):
                    pc = ps.tile([128, SKIP], F32, tag="pt")
                    for kt in range(4):
                        nc.tensor.matmul(pc, wq[:, kt, m * 128:(m + 1) * 128],
                                         A[:, kt, :],
                                         start=(kt == 0), stop=(kt == 3))
                    nc.scalar.copy(simf[:, m, :], pc)

                if stages < 4:
                    return
                # instance-norm stats over the whole 512x512 map
                statp = wts.tile([128, 8], F32, tag="statp")
                sqs = work.tile([128, SKIP], F32, tag="sqs")
                for m in range(4):
                    nc.vector.reduce_sum(statp[:, m:m + 1], simf[:, m, :], axis=AX.X)
                    nc.scalar.square(sqs, simf[:, m, :])
                    nc.vector.reduce_sum(statp[:, 4 + m:5 + m], sqs, axis=AX.X)
                srow = wts.tile([128, 2], F32, tag="srow")
                nc.vector.reduce_sum(srow[:, 0:1], statp[:, 0:4], axis=AX.X)
                nc.vector.reduce_sum(srow[:, 1:2], statp[:, 4:8], axis=AX.X)
                # partition -> free flip via tiny SBUF-to-SBUF DMA, then reduce
                flip = wts.tile([1, 2, 128], F32, tag="flip")
                for j in range(2):
                    nc.sync.dma_start(out=flip[:, j, :], in_=srow[:, j:j + 1])
                # scalars on partition 0
                sc = wts.tile([1, 8], F32, tag="sc")
                # sc cols: 0=s,1=q,2=mu,3=ex2,4=musq,5=var,6=sqrt,7=rsig
                epsT = wts.tile([1, 1], F32, tag="epsT")
                nc.vector.memset(epsT, IN_EPS)
                nc.vector.reduce_sum(sc[:, 0:1], flip[:, 0, :], axis=AX.X)
                nc.vector.reduce_sum(sc[:, 1:2], flip[:, 1, :], axis=AX.X)
                nc.scalar.mul(sc[:, 2:3], sc[:, 0:1], 1.0 / SIM_N)
                nc.scalar.mul(sc[:, 3:4], sc[:, 1:2], 1.0 / SIM_N)
                nc.vector.tensor_mul(sc[:, 4:5], sc[:, 2:3], sc[:, 2:3])
                nc.vector.tensor_sub(sc[:, 5:6], sc[:, 3:4], sc[:, 4:5])
                nc.scalar.activation(sc[:, 6:7], sc[:, 5:6], AF.Sqrt, bias=epsT)
                nc.vector.reciprocal(sc[:, 7:8], sc[:, 6:7])
                scal2 = wts.tile([1, 2], F32, tag="scal2")
                nc.vector.tensor_mul(scal2[:, 0:1], sc[:, 7:8], psi[:, 0:1])
                nc.scalar.mul(scal2[:, 1:2], scal2[:, 0:1], -1.0)
                # broadcast to all partitions via DRAM bounce
                nc.sync.dma_start(out=bc_scr_ap, in_=scal2)
                bcast_in = bass.AP(tensor=bc_scr_ap.tensor, offset=bc_scr_ap.offset,
                                   ap=[[0, 128], [1, 2]])
                bc = wts.tile([128, 2], F32, tag="bc")
                nc.sync.dma_start(out=bc, in_=bcast_in)

                if stages < 5:
                    return
                # softmax over t (free dim); psi_b cancels in softmax
                sm16 = wts.tile([128, 4, SKIP], F16, tag="sm16")
                for m in range(4):
                    rmax = work.tile([128, 1], F32, tag="rmax")
                    nc.vector.reduce_max(rmax, simf[:, m, :], axis=AX.X)
                    nm = work.tile([128, 1], F32, tag="nm")
                    nc.vector.tensor_mul(nm, rmax, bc[:, 1:2])
                    rsum = work.tile([128, 1], F32, tag="rsum")
                    nc.scalar.activation(simf[:, m, :], simf[:, m, :], AF.Exp,
                                         bias=nm, scale=bc[:, 0:1],
                                         accum_out=rsum)
                    rinv = work.tile([128, 1], F32, tag="rinv")
                    nc.vector.reciprocal(rinv, rsum)
                    nc.vector.tensor_scalar_mul(sm16[:, m, :], simf[:, m, :], rinv)

                if stages < 6:
                    return
                # G[t,o] = sum_s sm[s,t] wo[s,o]
                G = wts.tile([128, 4, SKIP], F16, tag="G")
                for m in range(4):
                    pg = ps.tile([128, SKIP], F32, tag="pt")
                    for kt in range(4):
                        nc.tensor.matmul(pg, sm16[:, kt, m * 128:(m + 1) * 128],
                                         wo[:, kt, :],
                                         start=(kt == 0), stop=(kt == 3))
                    nc.scalar.copy(G[:, m, :], pg)

                # recT[o,n] = sum_t G[t,o] vT[t,n]
                recT = wts.tile([128, 4, NPAT], F16, tag="recT")
                for m in range(4):
                    pr_ = ps.tile([128, NPAT], F32, tag="pt")
                    for kt in range(4):
                        nc.tensor.matmul(pr_, G[:, kt, m * 128:(m + 1) * 128],
                                         vT[:, kt, :],
                                         start=(kt == 0), stop=(kt == 3))
                    nc.scalar.copy(recT[:, m, :], pr_)

                # FIN = relu(rc'(recT) + b2')
                FIN = wts.tile([128, 4, NPAT], F32, tag="FIN")
                for m in range(4):
                    pf = ps.tile([128, NPAT], F32, tag="pt")
                    for kt in range(4):
                        nc.tensor.matmul(pf, rcT[:, kt, m * 128:(m + 1) * 128],
                                         recT[:, kt, :],
                                         start=(kt == 0), stop=(kt == 3))
                    nc.scalar.activation(FIN[:, m, :], pf, AF.Relu,
                                         bias=b2[:, m:m + 1])

                if stages < 7:
                    return
                # ---- stage 3: mask conv + recon multiply (patch-major) ----
                out_flat = out_d.rearrange("c a b -> c (a b)")
                W3 = 2 * NPAT
                for ppb in range(32):
                    p0 = ppb * W3
                    for m in range(4):
                        pM = ps.tile([128, W3], F32, tag="pt")
                        for kt in range(4):
                            nc.tensor.matmul(pM,
                                             mcT[:, kt, m * 128:(m + 1) * 128],
                                             dec_sb[kt][:, p0:p0 + W3],
                                             start=(kt == 0), stop=(kt == 3))
                        rl = st3.tile([128, W3], F32, tag="rl", bufs=6)
                        nc.scalar.activation(rl, pM, AF.Relu, bias=b1[:, m:m + 1])
                        ot = st3.tile([128, W3], F32, tag="ot", bufs=6)
                        fbase = FIN[:, m, :]
                        fb = bass.AP(tensor=fbase.tensor, offset=fbase.offset,
                                     ap=[fbase.ap[0], [0, 2], fbase.ap[1]])
                        nc.vector.tensor_mul(
                            ot.rearrange("p (a b) -> p a b", b=NPAT),
                            rl.rearrange("p (a b) -> p a b", b=NPAT), fb)
                        nc.sync.dma_start(
                            out=out_flat[m * 128:(m + 1) * 128, p0:p0 + W3],
                            in_=ot)

            if repeat == 1:
                body()
            else:
                with tc.For_i(0, repeat, 1):
                    body()
    nc.finalize()
    return nc


def prepare_in_maps(inputs: dict) -> list[dict]:
    f16 = np.float16
    decoder = np.asarray(inputs["decoder"], np.float32)
    trans = np.asarray(inputs["trans"], np.float32)
    pe_w = np.asarray(inputs["pe_w"], np.float32)
    pe_b = np.asarray(inputs["pe_b"], np.float32)
    mc_w = np.asarray(inputs["mc_w"], np.float32)
    mc_b = np.asarray(inputs["mc_b"], np.float32)
    bn1_g = np.asarray(inputs["bn1_g"], np.float32)
    bn1_b = np.asarray(inputs["bn1_b"], np.float32)
    bn1_m = np.asarray(inputs["bn1_m"], np.float32)
    bn1_v = np.asarray(inputs["bn1_v"], np.float32)
    wq = np.asarray(inputs["wq"], np.float32)
    wk = np.asarray(inputs["wk"], np.float32)
    wv = np.asarray(inputs["wv"], np.float32)
    wo = np.asarray(inputs["wo"], np.float32)
    psi_g = np.asarray(inputs["psi_g"], np.float32)
    psi_b = np.asarray(inputs["psi_b"], np.float32)
    rc_w = np.asarray(inputs["rc_w"], np.float32)
    rc_b = np.asarray(inputs["rc_b"], np.float32)
    bn2_g = np.asarray(inputs["bn2_g"], np.float32)
    bn2_b = np.asarray(inputs["bn2_b"], np.float32)
    bn2_m = np.asarray(inputs["bn2_m"], np.float32)
    bn2_v = np.asarray(inputs["bn2_v"], np.float32)

    s1 = bn1_g / np.sqrt(bn1_v + BN_EPS)
    mcT = np.ascontiguousarray((mc_w[:, :, 0, 0] * s1[:, None]).T)
    b1 = (mc_b - bn1_m) * s1 + bn1_b
    s2 = bn2_g / np.sqrt(bn2_v + BN_EPS)
    rcT = np.ascontiguousarray((rc_w[:, :, 0, 0] * s2[:, None]).T)
    b2 = (rc_b - bn2_m) * s2 + bn2_b

    pew = np.ascontiguousarray(
        pe_w.transpose(1, 2, 3, 0).reshape(4, 128, 64, DEC).transpose(0, 2, 1, 3)
    ).reshape(256, 128, DEC)

    shared = {
        "pew": pew.astype(f16),
        "wq": wq.astype(f16),
        "wk": wk.astype(f16),
        "wv": wv.astype(f16),
        "wo": wo.astype(f16),
        "mcT": mcT.astype(f16),
        "rcT": rcT.astype(f16),
        "peb": pe_b.reshape(1, DEC).astype(f16),
        "b1": np.ascontiguousarray(b1.reshape(4, 128).T).astype(np.float32),
        "b2": np.ascontiguousarray(b2.reshape(4, 128).T).astype(np.float32),
        "psi": np.array([[psi_g[0], psi_b[0]]], np.float32),
    }
    in_maps = []
    for c in range(N_CORES):
        m = dict(shared)
        m["dec"] = np.ascontiguousarray(
            decoder[c].reshape(CIN, NPR, P, NPR, P).transpose(0, 2, 4, 1, 3)
            .reshape(CIN, 64, NPAT)).astype(f16)
        m["transT"] = np.ascontiguousarray(trans[c].T).astype(f16)
        in_maps.append(m)
    return in_maps


_NC_CACHE: dict = {}


def get_nc(repeat: int = 1):
    if repeat not in _NC_CACHE:
        _NC_CACHE[repeat] = build_nc(repeat)
    return _NC_CACHE[repeat]


def kernel(**inputs) -> np.ndarray:
    nc = get_nc(1)
    in_maps = prepare_in_maps(inputs)
    res = run_bass_kernel_spmd(nc, in_maps, core_ids=list(range(N_CORES)))
    outs = []
    for c in range(N_CORES):
        oq = res.results[c]["out"].reshape(SKIP, P, P, NPR, NPR)
        outs.append(oq.transpose(0, 3, 1, 4, 2).reshape(SKIP, IMG, IMG))
    return np.stack(outs).astype(np.float32)


if __name__ == "__main__":
    import jax

    sys.path.insert(0, "/root/problem")
    import reference

    inputs = {k: np.asarray(v) for k, v in reference.setup_inputs().items()}
    out = kernel(**inputs)
    print("out shape", out.shape, out.dtype)



# revision 2
# speedup vs baseline: 1.0092x; 1.0092x over previous
"""Trainium2 Bass kernel for nn_DRA_C_65644280152592 (dense_transformer), v2.

Data-parallel over batch B=8 (one sample per core). Versus v1:
  - patch embed in [d, n] layout: stationary = pew k-chunk (128 cols, FWL),
    moving = resident dec slice (N=196). 1024 MMs instead of 512 N=512 MMs.
  - pew streamed as fp8_e4m3 (x32 prescale, folded out via wq/32): halves
    the dominant DMA stream. dec/everything else stays fp16.
  - batched DMAs: pew in 16 x 1.05MB groups (sync ring); dec + output on the
    scalar (ACT) HWDGE ring to decouple from the pew stream.
  - attention via QT path (no A intermediate, no transposes).
  - InstanceNorm stats: row sums via ACT accum_out + DVE square/reduce;
    partition reduce + scalar broadcast via tiny ones-matmuls (no DRAM
    bounce). Softmax without max-subtraction (post-IN logits are ~N(0,1)).
  - stage 3 (mask conv * FIN) emitted partially interleaved with the softmax
    chain to keep the PE warm; fp16 output, staged in [128,8,196] tiles and
    written in 802KB DMAs.
"""
import sys

sys.path.insert(0, "/opt/trn_rl_repo")

import numpy as np

import concourse.bass as bass
import concourse.mybir as mybir
import concourse.tile as tile
from concourse import bacc
from concourse.bass_utils import run_bass_kernel_spmd

F8 = mybir.dt.float8e4
F16 = mybir.dt.float16
F32 = mybir.dt.float32
AF = mybir.ActivationFunctionType
AX = mybir.AxisListType

CIN, IMG, P = 512, 112, 8
NPR = 14
NPAT = NPR * NPR          # 196
DEC = SKIP = 512
EMB = 768
BN_EPS = 1e-3
IN_EPS = 1e-3
N_CORES = 8
SIM_N = float(SKIP * SKIP)
PEW_SCALE = 32.0
PEW_FP8 = True

NG = 32                   # pew stream groups
KPG = 256 // NG           # k-chunks per group (8)


def build_nc(repeat: int = 1):
    nc = bacc.Bacc(None)

    dec_d = nc.declare_dram_parameter("dec", [CIN, 64, NPAT], F16, isOutput=False)
    pew_d = nc.declare_dram_parameter("pew", [128, 256, DEC],
                                      F8 if PEW_FP8 else F16, isOutput=False)
    trT_d = nc.declare_dram_parameter("trT", [128, 6, NPAT], F16, isOutput=False)
    wk_d = nc.declare_dram_parameter("wk", [128, 6, SKIP], F16, isOutput=False)
    wv_d = nc.declare_dram_parameter("wv", [128, 6, SKIP], F16, isOutput=False)
    wq_d = nc.declare_dram_parameter("wq", [128, 4, SKIP], F16, isOutput=False)
    wo_d = nc.declare_dram_parameter("wo", [128, 4, SKIP], F16, isOutput=False)
    mcT_d = nc.declare_dram_parameter("mcT", [128, 4, SKIP], F16, isOutput=False)
    rcT_d = nc.declare_dram_parameter("rcT", [128, 4, SKIP], F16, isOutput=False)
    peb_d = nc.declare_dram_parameter("peb", [1, DEC], F16, isOutput=False)
    b1_d = nc.declare_dram_parameter("b1", [128, 4], F32, isOutput=False)
    b2_d = nc.declare_dram_parameter("b2", [128, 4], F32, isOutput=False)
    psi_d = nc.declare_dram_parameter("psi", [1, 2], F32, isOutput=False)
    out_d = nc.declare_dram_parameter("out16", [SKIP, 64, NPAT], F16, isOutput=True)

    with tile.TileContext(nc) as tc:
        with tc.tile_pool(name="wts", bufs=1) as wts, \
             tc.tile_pool(name="pewp", bufs=6) as pewp, \
             tc.tile_pool(name="work", bufs=1) as work, \
             tc.tile_pool(name="rlp", bufs=16) as rlp, \
             tc.tile_pool(name="outst", bufs=3) as outst, \
             tc.tile_pool(name="psA", bufs=1, space="PSUM") as psA, \
             tc.tile_pool(name="psW", bufs=3, space="PSUM") as psW:

            def body():
                with nc.named_scope("setup"):
                    ones16 = wts.tile([1, NPAT], F16, tag="ones16")
                    nc.vector.memset(ones16, 1.0)
                    ones32r = wts.tile([1, 128], F32, tag="ones32r")
                    nc.vector.memset(ones32r, 1.0)
                    ones32c = wts.tile([128, 1], F32, tag="ones32c")
                    nc.vector.memset(ones32c, 1.0)
                    epsT = wts.tile([1, 1], F32, tag="epsT")
                    nc.vector.memset(epsT, IN_EPS)
                    # preload ACT tables for Sqrt/Exp so the loads don't land
                    # on the softmax critical path
                    tdum = wts.tile([1, 2], F32, tag="tdum")
                    nc.scalar.activation(tdum[:, 0:1], epsT, AF.Sqrt)
                    nc.scalar.activation(tdum[:, 1:2], epsT, AF.Exp)

                    dec_sb = [wts.tile([128, 64 * NPAT], F16, tag=f"dec{cb}",
                                       name=f"dec{cb}") for cb in range(4)]
                    # dec cb0 on the scalar ring: two eighth-loads first (the
                    # first one gates the very first patch-embed matmul),
                    # then three quarters
                    v0 = dec_sb[0].rearrange("p (a b) -> p a b", b=NPAT)
                    for e in range(2):
                        nc.scalar.dma_start(
                            out=v0[:, e * 8:(e + 1) * 8, :],
                            in_=dec_d[0:128, e * 8:(e + 1) * 8, :])
                    for q in range(1, 4):
                        nc.scalar.dma_start(
                            out=v0[:, q * 16:(q + 1) * 16, :],
                            in_=dec_d[0:128, q * 16:(q + 1) * 16, :])
                    peb = wts.tile([1, DEC], F16, tag="peb")
                    nc.gpsimd.dma_start(out=peb, in_=peb_d[:])
                    b1 = wts.tile([128, 4], F32, tag="b1")
                    nc.gpsimd.dma_start(out=b1, in_=b1_d[:])
                    b2 = wts.tile([128, 4], F32, tag="b2")
                    nc.gpsimd.dma_start(out=b2, in_=b2_d[:])
                    psi = wts.tile([1, 2], F32, tag="psi")
                    nc.gpsimd.dma_start(out=psi, in_=psi_d[:])

                    trT = wts.tile([128, 6, NPAT], F16, tag="trT")
                    wk = wts.tile([128, 6, SKIP], F16, tag="wk")
                    wv = wts.tile([128, 6, SKIP], F16, tag="wv")
                    wq = wts.tile([128, 4, SKIP], F16, tag="wq")
                    wo = wts.tile([128, 4, SKIP], F16, tag="wo")
                    mcT = wts.tile([128, 4, SKIP], F16, tag="mcT")
                    rcT = wts.tile([128, 4, SKIP], F16, tag="rcT")

                # hoisted stage-3 blocks (independent of attention) keep the
                # PE warm through DMA-starved stretches of phase A and the
                # attention serial chains. MM+ACT only; the multiplies and
                # out DMAs are emitted after FIN.
                hoisted = []

                def conv_block(m, g3, j, in_A=False):
                    # one [128, 392] mask-conv block: positions 8*g3+2j, +1
                    # in_A: the pdl psum tags are busy accumulating during
                    # phase A, so A-tail blocks use the psW rotation instead.
                    p0 = (8 * g3 + 2 * j) * NPAT
                    if in_A:
                        pM = psW.tile([128, SKIP], F32, tag="pt")
                    else:
                        pM = psA.tile([128, SKIP], F32, tag=f"pdl{j}")
                    for kt in range(4):
                        nc.tensor.matmul(pM[:, 0:2 * NPAT],
                                         mcT[:, kt, m * 128:(m + 1) * 128],
                                         dec_sb[kt][:, p0:p0 + 2 * NPAT],
                                         start=(kt == 0), stop=(kt == 3))
                    rl = rlp.tile([128, 2 * NPAT], F16, tag="rl")
                    nc.scalar.activation(rl, pM[:, 0:2 * NPAT], AF.Relu,
                                         bias=b1[:, m:m + 1])
                    return rl

                def mult_block(m, j, rl, ot, FIN16):
                    fbase = FIN16[:, m, :]
                    fb = bass.AP(tensor=fbase.tensor, offset=fbase.offset,
                                 ap=[fbase.ap[0], [0, 2], fbase.ap[1]])
                    nc.vector.tensor_mul(
                        ot[:, 2 * j:2 * j + 2, :],
                        rl.rearrange("p (a b) -> p a b", b=NPAT), fb)

                # ---- stage 1: patch embedding, dl[d, n] layout, with the
                # km/vT matmuls and late weight loads woven in ----
                with nc.named_scope("patch_embed"):
                    NSPL = (128, 68)
                    km = [wts.tile([NSPL[h], SKIP], F16, tag=f"km{h}",
                                   name=f"km{h}") for h in range(2)]
                    vT = wts.tile([128, 4, NPAT], F16, tag="vT")

                    pdl = [psA.tile([128, SKIP], F32, tag=f"pdl{d}",
                                    name=f"pdl{d}") for d in range(4)]
                    # dec cb1-3 half-loads interleaved between the pew odd
                    # groups on the scalar ring: FIFO order throttles them so
                    # they arrive just before the groups that need them
                    # (cb1 @ g8, cb2 @ g16, cb3 @ g24) without stealing HBM
                    # bandwidth from the critical early pew groups.
                    dec_after = {3: (1, 0), 5: (1, 1), 9: (2, 0),
                                 13: (2, 1), 17: (3, 0), 21: (3, 1)}
                    for g in range(NG):
                        pw = pewp.tile([128, KPG, DEC], F8 if PEW_FP8 else F16,
                                       tag="pw")
                        eng = nc.sync if g % 2 == 0 else nc.scalar
                        eng.dma_start(out=pw,
                                      in_=pew_d[:, g * KPG:(g + 1) * KPG, :])
                        if g == 0:
                            nc.sync.dma_start(out=trT, in_=trT_d[:])
                            nc.sync.dma_start(out=wk, in_=wk_d[:])
                            nc.sync.dma_start(out=wv, in_=wv_d[:])
                        if g in dec_after:
                            cb, half = dec_after[g]
                            v = dec_sb[cb].rearrange("p (a b) -> p a b", b=NPAT)
                            nc.scalar.dma_start(
                                out=v[:, half * 32:(half + 1) * 32, :],
                                in_=dec_d[cb * 128:(cb + 1) * 128,
                                          half * 32:(half + 1) * 32, :])
                        for j in range(KPG):
                            k = g * KPG + j
                            cb, pp = k // 64, k % 64
                            xs = dec_sb[cb][:, pp * NPAT:(pp + 1) * NPAT]
                            for d in range(4):
                                nc.tensor.matmul(
                                    pdl[d][:, 0:NPAT],
                                    pw[:, j, d * 128:(d + 1) * 128],
                                    xs, start=(k == 0), stop=False)
                        if g == 8:
                            # km = trans@wk (needs trT/wk, loaded by now)
                            for h in range(2):
                                n0 = 128 * h
                                pk = psW.tile([NSPL[h], SKIP], F32, tag="pt")
                                for kt in range(6):
                                    nc.tensor.matmul(
                                        pk, trT[:, kt, n0:n0 + NSPL[h]],
                                        wk[:, kt, :],
                                        start=(kt == 0), stop=(kt == 5))
                                nc.scalar.copy(km[h], pk)
                        elif g == 10:
                            # vT = wv^T@trans^T
                            for m in range(4):
                                pv = psW.tile([128, NPAT], F32, tag="pt")
                                for kt in range(6):
                                    nc.tensor.matmul(
                                        pv, wv[:, kt, m * 128:(m + 1) * 128],
                                        trT[:, kt, :],
                                        start=(kt == 0), stop=(kt == 5))
                                nc.scalar.copy(vT[:, m, :], pv)
                        elif g == 16:
                            nc.sync.dma_start(out=wq, in_=wq_d[:])
                            nc.sync.dma_start(out=wo, in_=wo_d[:])
                        elif g == 20:
                            nc.sync.dma_start(out=mcT, in_=mcT_d[:])
                            nc.sync.dma_start(out=rcT, in_=rcT_d[:])
                    # + pe_b via K=1 ones matmul, then copy to fp16
                    dl16 = wts.tile([128, 4, NPAT], F16, tag="dl16")
                    for d in range(4):
                        nc.tensor.matmul(pdl[d][:, 0:NPAT],
                                         peb[:, d * 128:(d + 1) * 128],
                                         ones16, start=False, stop=True)
                        nc.scalar.copy(dl16[:, d, :], pdl[d][:, 0:NPAT])

                with nc.named_scope("hoist_0"):
                    for j in range(4):
                        hoisted.append((0, 0, j, conv_block(0, 0, j)))

                # ---- stage 2: attention ----
                with nc.named_scope("attn_sim"):
                    NSPL = (128, 68)
                    # QT[n, s] = sum_d dl[d, n] wq'[d, s]
                    QT = [wts.tile([NSPL[h], SKIP], F16, tag=f"QT{h}",
                                   name=f"QT{h}") for h in range(2)]
                    for h in range(2):
                        n0 = 128 * h
                        pq = psW.tile([NSPL[h], SKIP], F32, tag="pt")
                        for d in range(4):
                            nc.tensor.matmul(pq,
                                             dl16[:, d, n0:n0 + NSPL[h]],
                                             wq[:, d, :],
                                             start=(d == 0), stop=(d == 3))
                        nc.scalar.copy(QT[h], pq)

                    # sim[s, t] = sum_n QT[n, s] km[n, t]
                    simf = wts.tile([128, 4, SKIP], F32, tag="simf")
                    statp = wts.tile([128, 8], F32, tag="statp")
                    for m in range(4):
                        pc = psW.tile([128, SKIP], F32, tag="pt")
                        for h in range(2):
                            nc.tensor.matmul(pc,
                                             QT[h][:, m * 128:(m + 1) * 128],
                                             km[h],
                                             start=(h == 0), stop=(h == 1))
                        # psum -> SBUF copy with free row-sum; squares on DVE
                        nc.scalar.activation(simf[:, m, :], pc, AF.Identity,
                                             accum_out=statp[:, m:m + 1])
                        sq = work.tile([128, SKIP], F32, tag="sqs",
                                       name=f"sqs{m}")
                        nc.vector.tensor_mul(sq, simf[:, m, :], simf[:, m, :])
                        nc.vector.reduce_sum(statp[:, 4 + m:5 + m], sq,
                                             axis=AX.X)

                with nc.named_scope("hoist_a"):
                    for j in range(4):
                        hoisted.append((0, 1, j, conv_block(0, 1, j)))

                # ---- instance-norm stats ----
                with nc.named_scope("in_stats"):
                    pst = psW.tile([128, SKIP], F32, tag="pt")
                    nc.tensor.matmul(pst[0:1, 0:8], ones32c, statp,
                                     start=True, stop=True)
                    sc8 = wts.tile([1, 8], F32, tag="sc8")
                    nc.scalar.copy(sc8, pst[0:1, 0:8])
                    sc = wts.tile([1, 8], F32, tag="sc")
                    # cols: 0=S1, 1=S2, 2=-mu, 3=ex2, 4=mu^2, 5=var, 6=sig
                    nc.vector.reduce_sum(sc[:, 0:1], sc8[:, 0:4], axis=AX.X)
                    nc.vector.reduce_sum(sc[:, 1:2], sc8[:, 4:8], axis=AX.X)
                    nc.scalar.mul(sc[:, 2:3], sc[:, 0:1], -1.0 / SIM_N)
                    nc.scalar.mul(sc[:, 3:4], sc[:, 1:2], 1.0 / SIM_N)
                    nc.vector.tensor_mul(sc[:, 4:5], sc[:, 2:3], sc[:, 2:3])
                    nc.vector.tensor_sub(sc[:, 5:6], sc[:, 3:4], sc[:, 4:5])
                    nc.scalar.activation(sc[:, 6:7], sc[:, 5:6], AF.Sqrt,
                                         bias=epsT)
                    scal2 = wts.tile([1, 2], F32, tag="scal2")
                    nc.vector.reciprocal(scal2[:, 0:1], sc[:, 6:7])
                    nc.vector.tensor_mul(scal2[:, 0:1], scal2[:, 0:1],
                                         psi[:, 0:1])
                    nc.vector.tensor_mul(scal2[:, 1:2], scal2[:, 0:1],
                                         sc[:, 2:3])
                    # broadcast (scale, -mu*scale) to all 128 partitions
                    pbc = psW.tile([128, SKIP], F32, tag="pt")
                    nc.tensor.matmul(pbc[:, 0:2], ones32r, scal2,
                                     start=True, stop=True)
                    bc = wts.tile([128, 2], F32, tag="bc")
                    nc.scalar.copy(bc, pbc[:, 0:2])

                with nc.named_scope("hoist_b"):
                    for j in range(4):
                        hoisted.append((0, 2, j, conv_block(0, 2, j)))

                # ---- softmax over t (psi_b cancels) ----
                with nc.named_scope("softmax"):
                    rsum = wts.tile([128, 4], F32, tag="rsum")
                    rinv = wts.tile([128, 4], F32, tag="rinv")
                    sm16 = wts.tile([128, 4, SKIP], F16, tag="sm16")
                    for m in range(4):
                        nc.scalar.activation(simf[:, m, :], simf[:, m, :],
                                             AF.Exp, bias=bc[:, 1:2],
                                             scale=bc[:, 0:1],
                                             accum_out=rsum[:, m:m + 1])
                        nc.vector.reciprocal(rinv[:, m:m + 1], rsum[:, m:m + 1])
                        nc.vector.tensor_scalar_mul(sm16[:, m, :],
                                                    simf[:, m, :],
                                                    rinv[:, m:m + 1])

                # ---- G = sm^T wo ; recT = G^T vT ; FIN = relu(rc(recT)+b2) ----
                with nc.named_scope("attn_out"):
                    G16 = wts.tile([128, 4, SKIP], F16, tag="G16")
                    for m in range(4):
                        pg = psW.tile([128, SKIP], F32, tag="pt")
                        for kt in range(4):
                            nc.tensor.matmul(pg,
                                             sm16[:, kt, m * 128:(m + 1) * 128],
                                             wo[:, kt, :],
                                             start=(kt == 0), stop=(kt == 3))
                        nc.scalar.copy(G16[:, m, :], pg)

                    recT = wts.tile([128, 4, NPAT], F16, tag="recT")
                    for m in range(4):
                        pr_ = psW.tile([128, NPAT], F32, tag="pt")
                        for kt in range(4):
                            nc.tensor.matmul(pr_,
                                             G16[:, kt, m * 128:(m + 1) * 128],
                                             vT[:, kt, :],
                                             start=(kt == 0), stop=(kt == 3))
                        nc.scalar.copy(recT[:, m, :], pr_)

                    FIN16 = wts.tile([128, 4, NPAT], F16, tag="FIN16")
                    for m in range(4):
                        pf = psW.tile([128, NPAT], F32, tag="pt")
                        for kt in range(4):
                            nc.tensor.matmul(pf,
                                             rcT[:, kt, m * 128:(m + 1) * 128],
                                             recT[:, kt, :],
                                             start=(kt == 0), stop=(kt == 3))
                        nc.scalar.activation(FIN16[:, m, :], pf, AF.Relu,
                                             bias=b2[:, m:m + 1])

                # ---- stage 3: mask conv + multiply + fp16 out ----
                with nc.named_scope("stage3"):
                    # finish the hoisted blocks (m=0, g3=0..2)
                    groups = {}
                    for (m, g3, j, rl) in hoisted:
                        groups.setdefault((m, g3), []).append((j, rl))
                    for (m, g3), blocks in sorted(groups.items()):
                        ot = outst.tile([128, 8, NPAT], F16, tag="ot")
                        for j, rl in sorted(blocks):
                            mult_block(m, j, rl, ot, FIN16)
                        eng = nc.sync if g3 % 2 == 0 else nc.scalar
                        eng.dma_start(
                            out=out_d[m * 128:(m + 1) * 128,
                                      g3 * 8:(g3 + 1) * 8, :], in_=ot)
                    for m in range(4):
                        for g3 in range(3 if m == 0 else 0, 8):
                            ot = outst.tile([128, 8, NPAT], F16, tag="ot")
                            for j in range(4):
                                rl = conv_block(m, g3, j)
                                mult_block(m, j, rl, ot, FIN16)
                            if m == 3 and g3 >= 6:
                                # split the final groups across both rings to
                                # shorten the end-of-kernel write drain
                                nc.sync.dma_start(
                                    out=out_d[m * 128:(m + 1) * 128,
                                              g3 * 8:g3 * 8 + 4, :],
                                    in_=ot[:, 0:4, :])
                                nc.scalar.dma_start(
                                    out=out_d[m * 128:(m + 1) * 128,
                                              g3 * 8 + 4:(g3 + 1) * 8, :],
                                    in_=ot[:, 4:8, :])
                            else:
                                eng = (nc.sync if (m * 8 + g3) % 2 == 0
                                       else nc.scalar)
                                eng.dma_start(
                                    out=out_d[m * 128:(m + 1) * 128,
                                              g3 * 8:(g3 + 1) * 8, :], in_=ot)

            if repeat == 1:
                body()
            else:
                with tc.For_i(0, repeat, 1):
                    body()
    nc.finalize()
    return nc


def prepare_in_maps(inputs: dict) -> list[dict]:
    f16 = np.float16
    import ml_dtypes
    f8 = ml_dtypes.float8_e4m3
    decoder = np.asarray(inputs["decoder"], np.float32)
    trans = np.asarray(inputs["trans"], np.float32)
    pe_w = np.asarray(inputs["pe_w"], np.float32)
    pe_b = np.asarray(inputs["pe_b"], np.float32)
    mc_w = np.asarray(inputs["mc_w"], np.float32)
    mc_b = np.asarray(inputs["mc_b"], np.float32)
    bn1_g = np.asarray(inputs["bn1_g"], np.float32)
    bn1_b = np.asarray(inputs["bn1_b"], np.float32)
    bn1_m = np.asarray(inputs["bn1_m"], np.float32)
    bn1_v = np.asarray(inputs["bn1_v"], np.float32)
    wq = np.asarray(inputs["wq"], np.float32)
    wk = np.asarray(inputs["wk"], np.float32)
    wv = np.asarray(inputs["wv"], np.float32)
    wo = np.asarray(inputs["wo"], np.float32)
    psi_g = np.asarray(inputs["psi_g"], np.float32)
    psi_b = np.asarray(inputs["psi_b"], np.float32)
    rc_w = np.asarray(inputs["rc_w"], np.float32)
    rc_b = np.asarray(inputs["rc_b"], np.float32)
    bn2_g = np.asarray(inputs["bn2_g"], np.float32)
    bn2_b = np.asarray(inputs["bn2_b"], np.float32)
    bn2_m = np.asarray(inputs["bn2_m"], np.float32)
    bn2_v = np.asarray(inputs["bn2_v"], np.float32)

    s1 = bn1_g / np.sqrt(bn1_v + BN_EPS)
    mcT = (mc_w[:, :, 0, 0] * s1[:, None]).T          # [cin, out]
    b1 = (mc_b - bn1_m) * s1 + bn1_b
    s2 = bn2_g / np.sqrt(bn2_v + BN_EPS)
    rcT = (rc_w[:, :, 0, 0] * s2[:, None]).T
    b2 = (rc_b - bn2_m) * s2 + bn2_b

    def chunk(w, kchunks):                            # [K, N] -> [128, kc, N]
        K, N = w.shape
        return np.ascontiguousarray(
            w.reshape(kchunks, 128, N).transpose(1, 0, 2))

    # pew: [d_out, c, py, px] -> [cp, k=(cb,pp), d]
    pew = (pe_w.transpose(1, 2, 3, 0)                 # [c, py, px, d]
           .reshape(4, 128, 64, DEC)                  # [cb, cp, pp, d]
           .transpose(1, 0, 2, 3)                     # [cp, cb, pp, d]
           .reshape(128, 256, DEC))
    if PEW_FP8:
        pew_st = np.ascontiguousarray(pew * PEW_SCALE).astype(f8)
        wq_st = chunk(wq / PEW_SCALE, 4).astype(f16)
        peb_st = (pe_b.reshape(1, DEC) * PEW_SCALE).astype(f16)
    else:
        pew_st = np.ascontiguousarray(pew).astype(f16)
        wq_st = chunk(wq, 4).astype(f16)
        peb_st = pe_b.reshape(1, DEC).astype(f16)

    shared = {
        "pew": pew_st,
        "wq": wq_st,
        "wk": chunk(wk, 6).astype(f16),
        "wv": chunk(wv, 6).astype(f16),
        "wo": chunk(wo, 4).astype(f16),
        "mcT": chunk(mcT, 4).astype(f16),
        "rcT": chunk(rcT, 4).astype(f16),
        "peb": peb_st,
        "b1": np.ascontiguousarray(b1.reshape(4, 128).T).astype(np.float32),
        "b2": np.ascontiguousarray(b2.reshape(4, 128).T).astype(np.float32),
        "psi": np.array([[psi_g[0], psi_b[0]]], np.float32),
    }
    in_maps = []
    for c in range(N_CORES):
        m = dict(shared)
        m["dec"] = np.ascontiguousarray(
            decoder[c].reshape(CIN, NPR, P, NPR, P).transpose(0, 2, 4, 1, 3)
            .reshape(CIN, 64, NPAT)).astype(f16)
        m["trT"] = chunk(np.ascontiguousarray(trans[c].T), 6).astype(f16)
        in_maps.append(m)
    return in_maps


_NC_CACHE: dict = {}


def get_nc(repeat: int = 1):
    if repeat not in _NC_CACHE:
        _NC_CACHE[repeat] = build_nc(repeat)
    return _NC_CACHE[repeat]


def kernel(**inputs) -> np.ndarray:
    nc = get_nc(1)
    in_maps = prepare_in_maps(inputs)
    res = run_bass_kernel_spmd(nc, in_maps, core_ids=list(range(N_CORES)))
    outs = []
    for c in range(N_CORES):
        oq = res.results[c]["out16"].astype(np.float32).reshape(
            SKIP, P, P, NPR, NPR)
        outs.append(oq.transpose(0, 3, 1, 4, 2).reshape(SKIP, IMG, IMG))
    return np.stack(outs).astype(np.float32)


if __name__ == "__main__":
    import jax

    sys.path.insert(0, "/root/problem")
    import reference

    inputs = {k: np.asarray(v) for k, v in reference.setup_inputs().items()}
    out = kernel(**inputs)
    print("out shape", out.shape, out.dtype)
